# revision 11
# baseline (speedup 1.0000x reference)
"""Trainium2 Bass kernel for nn_Criterion_82738249990265.

Strategy (data-parallel over the 8 sentences, one per NeuronCore):
  - Host precomputes the tiny per-sentence word gates g0/g1 (tanh of 128x128
    matvecs) and the mask-normalization weights w0/w1 (3x3 box conv on the
    64x64 mask) in numpy.
  - Each core computes, for its sentence i and all 8 videos:
        C1 = conv3x3(v_map, Wc0, pad=2)                  (PE, fp32r matmuls)
        x  = relu(g0*C1 + g0*bc0) * w0                   (ACT relu + DVE mult)
        C2 = conv3x3(x, Wc1, pad=0)                      (PE)
        z  = relu(g1*C2 + g1*bc1)                        (ACT)
        L  = w2^T z                                      (PE, M=1 matmul)
    and writes the raw logits L [8, 64, 64] to DRAM.
  - Host applies w1, bias, sigmoid, validity masks (-inf), per-video maxes,
    the IoU-based hard-negative mining, and the top-k triplet loss — all on
    [8,8] / [8,4096]-sized data.
  - Conv work is restricted at compile time to the spatial blocks actually
    needed given the validity masks (upper-triangular v_mask + valid_num
    column limits), cutting ~60% of the dense FLOPs.
"""
import numpy as np
from contextlib import ExitStack

import concourse.bass as bass
import concourse.bacc as bacc
import concourse.mybir as mybir
import concourse.tile as tile
from concourse import bass_utils

F32 = mybir.dt.float32
F32R = mybir.dt.float32r

# problem constants (hardcoded per contract)
B = 8       # videos == sentences
C = 128     # channels
D = 64      # rows
T = 64      # cols
P = D * T
KK = 3      # conv kernel
FIRST_PAD = 2
H1 = D + 2 * FIRST_PAD - KK + 1   # 66, conv1 output spatial
HP = D + 2 * FIRST_PAD            # 68, padded input spatial
MARGIN = 0.1
NEG_NUM = 3
MAX_N = 512  # PSUM bank limit (fp32)

_KERNEL_CACHE = {}


def _conv2d_np(x, w, pad):
    Bn, Cn, H, W = x.shape
    O, I, kh, kw = w.shape
    xp = np.zeros((Bn, Cn, H + 2 * pad, W + 2 * pad), np.float32)
    xp[:, :, pad:pad + H, pad:pad + W] = x
    Ho, Wo = H + 2 * pad - kh + 1, W + 2 * pad - kw + 1
    out = np.zeros((Bn, O, Ho, Wo), np.float32)
    for ky in range(kh):
        for kx in range(kw):
            lhsT = w[:, :, ky, kx].T.astype(np.float32)
            patch = xp[:, :, ky:ky + Ho, kx:kx + Wo].reshape(Bn, Cn, Ho * Wo)
            out += np.einsum("io,bip->bop", lhsT, patch,
                             dtype=np.float32).reshape(Bn, O, Ho, Wo)
    return out


def _mask2weight_np(m, pad):
    ker = np.ones((1, 1, KK, KK), np.float32)
    w = _conv2d_np(m.astype(np.float32)[None, None], ker, pad)[0, 0]
    return np.where(w > 0, 1.0 / np.maximum(w, 1.0), 0.0).astype(np.float32)


def _row_ranges_from_mask(mask2d):
    """Per-row [lo, hi) of True entries (conservative contiguous span)."""
    out = []
    for r in range(mask2d.shape[0]):
        idx = np.nonzero(mask2d[r])[0]
        if len(idx) == 0:
            out.append(None)
        else:
            out.append((int(idx[0]), int(idx[-1]) + 1))
    return out


def _build_blocks(row_ranges, limit, max_n=MAX_N):
    """Greedy grouping of consecutive non-empty rows into blocks
    (r0, r1, lo, hi) with (r1-r0)*(hi-lo) <= max_n and even width
    (fp32r matmul requires even innermost counts)."""
    blocks = []
    n = len(row_ranges)
    r = 0
    while r < n:
        if row_ranges[r] is None:
            r += 1
            continue
        lo, hi = row_ranges[r]
        h = 1
        while r + h < n and row_ranges[r + h] is not None:
            nlo = min(lo, row_ranges[r + h][0])
            nhi = max(hi, row_ranges[r + h][1])
            w = nhi - nlo + ((nhi - nlo) % 2)
            if (h + 1) * w > max_n:
                break
            lo, hi, h = nlo, nhi, h + 1
        if (hi - lo) % 2:
            if hi < limit:
                hi += 1
            else:
                lo -= 1
        blocks.append((r, r + h, lo, hi))
        r += h
    return blocks


def _build_structure(vm3, w0):
    """Per-video conv2 blocks and conv1 (x-plane) blocks."""
    conv2_blocks = []   # per video: list of (r0, r1, lo, hi)
    conv1_blocks = []
    for j in range(B):
        rr = _row_ranges_from_mask(vm3[j])
        c2b = _build_blocks(rr, T)
        conv2_blocks.append(c2b)
        # x rows needed: conv2 block (r0,r1,lo,hi) reads x rows [r0, r1+2),
        # cols [lo, hi+2)
        xcov = [None] * H1
        for (r0, r1, lo, hi) in c2b:
            for rx in range(r0, min(r1 + KK - 1, H1)):
                lo2, hi2 = lo, min(hi + KK - 1, H1)
                if xcov[rx] is None:
                    xcov[rx] = (lo2, hi2)
                else:
                    xcov[rx] = (min(xcov[rx][0], lo2), max(xcov[rx][1], hi2))
        conv1_blocks.append(_build_blocks(xcov, H1))
    return conv2_blocks, conv1_blocks


def _build_bass(conv2_blocks, conv1_blocks, use_f32r=True):
    """Build and compile the SPMD Bass module (same program on all cores)."""
    nc = bacc.Bacc("TRN2", target_bir_lowering=False, debug=False)

    RD = F32R if use_f32r else F32

    # pre-padded on host: [B, C, 68, 68] with zero borders (pad=2)
    vmap_d = nc.dram_tensor("vmap", [B, C, HP, HP], RD, kind="ExternalInput")
    wc0_d = nc.dram_tensor("wc0t", [9, C, C], F32, kind="ExternalInput")
    wc1_d = nc.dram_tensor("wc1t", [9, C, C], F32, kind="ExternalInput")
    w2_d = nc.dram_tensor("w2", [C, 1], F32, kind="ExternalInput")
    g0_d = nc.dram_tensor("g0", [C, 1], F32, kind="ExternalInput")
    g1_d = nc.dram_tensor("g1", [C, 1], F32, kind="ExternalInput")
    g0b0_d = nc.dram_tensor("g0b0", [C, 1], F32, kind="ExternalInput")
    g1b1_d = nc.dram_tensor("g1b1", [C, 1], F32, kind="ExternalInput")
    w0_d = nc.dram_tensor("w0row", [1, H1 * H1], F32, kind="ExternalInput")
    ones_d = nc.dram_tensor("ones1", [1, C], F32, kind="ExternalInput")

    out_d = nc.dram_tensor("logits", [B, D, T], F32, kind="ExternalOutput")

    with tile.TileContext(nc) as tc:
        with ExitStack() as ctx:
            const = ctx.enter_context(tc.tile_pool(name="const", bufs=1))
            inp = ctx.enter_context(tc.tile_pool(name="inp", bufs=2))
            xpl = ctx.enter_context(tc.tile_pool(name="xpl", bufs=2))
            work = ctx.enter_context(tc.tile_pool(name="work", bufs=3))
            lrow = ctx.enter_context(tc.tile_pool(name="lrow", bufs=3))
            ps1 = ctx.enter_context(tc.tile_pool(name="ps1", bufs=3, space="PSUM"))
            ps2 = ctx.enter_context(tc.tile_pool(name="ps2", bufs=3, space="PSUM"))
            psL = ctx.enter_context(tc.tile_pool(name="psL", bufs=2, space="PSUM"))

            # ---- constants ----
            wc0_st = const.tile([C, 9 * C], F32)
            wc1_st = const.tile([C, 9 * C], F32)
            # dram [9, Cin, Cout]: one DMA per tap offset
            for off in range(9):
                nc.sync.dma_start(
                    wc0_st[:, off * C:(off + 1) * C], wc0_d[off])
                nc.sync.dma_start(
                    wc1_st[:, off * C:(off + 1) * C], wc1_d[off])
            g0_t = const.tile([C, 1], F32)
            g1_t = const.tile([C, 1], F32)
            g0b0_t = const.tile([C, 1], F32)
            g1b1_t = const.tile([C, 1], F32)
            w2_st = const.tile([C, 1], F32)
            ones_t = const.tile([1, C], F32)
            w0_t = const.tile([1, H1 * H1], F32)
            nc.sync.dma_start(g0_t[:], g0_d[:])
            nc.sync.dma_start(g1_t[:], g1_d[:])
            nc.sync.dma_start(g0b0_t[:], g0b0_d[:])
            nc.sync.dma_start(g1b1_t[:], g1b1_d[:])
            nc.sync.dma_start(w2_st[:], w2_d[:])
            nc.sync.dma_start(ones_t[:], ones_d[:])
            nc.sync.dma_start(w0_t[:], w0_d[:])

            if use_f32r:
                wc0_t = const.tile([C, 9 * C], F32R)
                wc1_t = const.tile([C, 9 * C], F32R)
                w2_t = const.tile([C, 1], F32R)
                nc.vector.tensor_copy(wc0_t[:], wc0_st[:])
                nc.vector.tensor_copy(wc1_t[:], wc1_st[:])
                nc.vector.tensor_copy(w2_t[:], w2_st[:])
            else:
                wc0_t, wc1_t, w2_t = wc0_st, wc1_st, w2_st

            # W0B: broadcast w0 row across 128 partitions via K=1 fp32 matmul
            w0b_t = const.tile([C, H1 * H1], F32)
            CH = 484  # 9 chunks of 484 = 4356
            for ci in range(9):
                pb = ps1.tile([C, CH], F32, tag="ps1")
                nc.tensor.matmul(pb[:], ones_t[:],
                                 w0_t[:, ci * CH:(ci + 1) * CH],
                                 start=True, stop=True)
                nc.any.tensor_copy(w0b_t[:, ci * CH:(ci + 1) * CH], pb[:])
            w0b3 = w0b_t[:].rearrange("p (a b) -> p a b", a=H1)

            # ---- per-video pipeline ----
            for j in range(B):
                c1b = conv1_blocks[j]
                c2b = conv2_blocks[j]
                if not c2b:
                    continue
                # padded input [C, 68, 68] (padding baked in on host)
                ipad = inp.tile([C, HP, HP], RD, tag="ipad")
                nc.sync.dma_start(ipad[:], vmap_d[j])

                # conv1 + gate -> x plane [C, 66, 66] (f32r)
                xp_t = xpl.tile([C, H1, H1], RD, tag="xplane")
                for (a0, a1, lo, hi) in c1b:
                    h, w = a1 - a0, hi - lo
                    p1 = ps1.tile([C, h, w], F32, tag="ps1")
                    for off in range(9):
                        ky, kx = off // 3, off % 3
                        nc.tensor.matmul(
                            p1[:],
                            wc0_t[:, off * C:(off + 1) * C],
                            ipad[:, a0 + ky:a0 + ky + h, lo + kx:lo + kx + w],
                            start=(off == 0), stop=(off == 8))
                    xr = work.tile([C, h, w], F32, tag="xr")
                    nc.scalar.activation(
                        xr[:], p1[:], mybir.ActivationFunctionType.Relu,
                        bias=g0b0_t[:], scale=g0_t[:])
                    nc.vector.tensor_mul(
                        xp_t[:, a0:a1, lo:hi], xr[:],
                        w0b3[:, a0:a1, lo:hi])

                # conv2 + gate + 1x1 conv -> logits
                for (r0, r1, lo, hi) in c2b:
                    h, w = r1 - r0, hi - lo
                    p2 = ps2.tile([C, h, w], F32, tag="ps2")
                    for off in range(9):
                        ky, kx = off // 3, off % 3
                        nc.tensor.matmul(
                            p2[:],
                            wc1_t[:, off * C:(off + 1) * C],
                            xp_t[:, r0 + ky:r0 + ky + h, lo + kx:lo + kx + w],
                            start=(off == 0), stop=(off == 8))
                    z = work.tile([C, h, w], RD, tag="z")
                    nc.scalar.activation(
                        z[:], p2[:], mybir.ActivationFunctionType.Relu,
                        bias=g1b1_t[:], scale=g1_t[:])
                    pl = psL.tile([1, h * w], F32, tag="psL")
                    nc.tensor.matmul(
                        pl[:], w2_t[:],
                        z[:].rearrange("p a b -> p (a b)"),
                        start=True, stop=True)
                    lr = lrow.tile([1, h, w], F32, tag="lrow")
                    nc.any.tensor_copy(
                        lr[:], pl[:].rearrange("p (a b) -> p a b", a=h))
                    nc.sync.dma_start(out_d[j, r0:r1, lo:hi], lr[:])

    nc.compile()
    return nc


def kernel(**inputs):
    v_map = np.ascontiguousarray(np.asarray(inputs["v_map"], np.float32))
    words = np.asarray(inputs["words"], np.float32)
    w_masks = np.asarray(inputs["w_masks"], np.float32)
    v_mask = np.asarray(inputs["v_mask"], np.float32)
    valid_num = np.asarray(inputs["valid_num"])
    iou_maps = np.asarray(inputs["iou_maps"], np.float32)
    lam = np.float32(inputs["lam"])
    Wc0 = np.asarray(inputs["Wc0"], np.float32)
    bc0 = np.asarray(inputs["bc0"], np.float32)
    Wc1 = np.asarray(inputs["Wc1"], np.float32)
    bc1 = np.asarray(inputs["bc1"], np.float32)
    Wp0 = np.asarray(inputs["Wp0"], np.float32)
    bp0 = np.asarray(inputs["bp0"], np.float32)
    Wp1 = np.asarray(inputs["Wp1"], np.float32)
    bp1 = np.asarray(inputs["bp1"], np.float32)
    W2d = np.asarray(inputs["W2d"], np.float32)
    b2d = np.asarray(inputs["b2d"], np.float32)

    # ---- host precompute (tiny) ----
    w0 = _mask2weight_np(v_mask, FIRST_PAD)               # [66,66]
    w1 = _mask2weight_np((w0 > 0).astype(np.float32), 0)  # [64,64]
    wm_sum = np.maximum(w_masks.sum(1), 1.0)
    pooled = (words * w_masks[:, :, None]).sum(1) / wm_sum[:, None]
    g0 = np.tanh(pooled @ Wp0 + bp0).astype(np.float32)   # [8,128]
    g1 = np.tanh(pooled @ Wp1 + bp1).astype(np.float32)
    w2 = W2d[0, :, 0, 0].astype(np.float32)               # [128]
    cols = np.arange(T)
    vm3 = (v_mask[None] > 0) & (cols[None, None, :] < valid_num[:, None, None])

    conv2_blocks, conv1_blocks = _build_structure(vm3, w0)

    # compile (cached on the block structure, which depends only on masks)
    skey = (str(conv2_blocks), str(conv1_blocks))
    if skey not in _KERNEL_CACHE:
        _KERNEL_CACHE[skey] = _build_bass(conv2_blocks, conv1_blocks)
    nc = _KERNEL_CACHE[skey]

    wc0t = np.ascontiguousarray(
        np.transpose(Wc0, (2, 3, 1, 0)).reshape(9, C, C).astype(np.float32))
    wc1t = np.ascontiguousarray(
        np.transpose(Wc1, (2, 3, 1, 0)).reshape(9, C, C).astype(np.float32))

    vm_pad = np.zeros((B, C, HP, HP), np.float32)
    vm_pad[:, :, FIRST_PAD:FIRST_PAD + D, FIRST_PAD:FIRST_PAD + T] = \
        v_map.reshape(B, C, D, T)

    base = dict(
        vmap=vm_pad,
        wc0t=wc0t, wc1t=wc1t,
        w2=w2.reshape(C, 1),
        w0row=w0.reshape(1, H1 * H1),
        ones1=np.ones((1, C), np.float32),
    )
    in_maps = []
    for i in range(B):
        m = dict(base)
        m["g0"] = np.ascontiguousarray(g0[i].reshape(C, 1))
        m["g1"] = np.ascontiguousarray(g1[i].reshape(C, 1))
        m["g0b0"] = np.ascontiguousarray((g0[i] * bc0).reshape(C, 1))
        m["g1b1"] = np.ascontiguousarray((g1[i] * bc1).reshape(C, 1))
        in_maps.append(m)

    res = bass_utils.run_bass_kernel_spmd(nc, in_maps, core_ids=list(range(B)))

    # ---- host post-processing ----
    w1f = w1.reshape(P)
    neg_inf = np.float32(-np.inf)
    scores = np.zeros((B, B), np.float32)
    neg_losses = np.zeros(B, np.float32)
    pmap = np.zeros((B, D, T), np.float32)
    for i in range(B):
        Lr = res.results[i]["logits"].reshape(B, P)
        s = (1.0 / (1.0 + np.exp(-(Lr * w1f[None, :] + b2d[0])))).astype(
            np.float32)
        s = np.where(vm3.reshape(B, P), s, neg_inf)
        pmap[i] = s[i].reshape(D, T)
        pos = s.max(1)
        scores[i] = pos
        si = s[i]
        pos_idx = int(np.argmax(si))
        iou_row = iou_maps[pos_idx]
        negs = np.where((iou_row < lam) & np.isfinite(si), si, neg_inf)
        hard = negs.max()
        neg_losses[i] = max(MARGIN + hard - pos[i], np.float32(0.0))

    negative_loss = neg_losses.mean(dtype=np.float32)
    diag = np.diag(scores)[:, None]
    cost_s = np.maximum(MARGIN + scores - diag, 0).astype(np.float32)
    cost_im = np.maximum(MARGIN + scores - diag.T, 0).astype(np.float32)
    eye = np.eye(B, dtype=bool)
    cost_s[eye] = 0.0
    cost_im[eye] = 0.0
    cost_s = -np.sort(-cost_s, axis=1)[:, :NEG_NUM]
    cost_im = -np.sort(-cost_im.T, axis=1)[:, :NEG_NUM]
    loss = np.float32(
        cost_s.sum() / B + cost_im.sum() / B + negative_loss)
    return loss, pmap
